# revision 17
# baseline (speedup 1.0000x reference)
"""Trainium2 Bass kernel for masked-GRU + residual + LayerNorm.

Problem: N=128 sequences of length L=512, hidden H=512.
  gx = x @ W_ih.T + b_ih            (precomputable input projection)
  per step l: hc = h * (1-is_initial[l]); gh = hc @ W_hh.T + b_hh
    r = sig(gx_r+gh_r); z = sig(gx_z+gh_z); n = tanh(gx_n + r*gh_n)
    h = (1-z)*n + z*hc
  out = LayerNorm(seq + x) * gamma + beta;  h_exp = broadcast(h_last)

Strategy:
  * Data parallel: 16 batch rows per core (8 cores).
  * Sequence-chunk parallel: each L=512 sequence is split into C=16
    chunks of 32 steps. Chunks are processed as independent columns,
    made exact by an R-step warm-up: the state entering chunk c only
    depends on inputs back to the latest reset (is_initial==1), and the
    data has a reset within every 16-step window (verified at runtime;
    R grows if needed). Chunk 0 injects the true h0 at its first step.
  * Per-core per-step state: hT [H=512 (4 partition tiles), 256 cols].
    Weights stationary, moving free dim 256, all matmul operands bf16
    (1 cycle/row on PE, same as f32r, but 2x DVE elementwise and half
    the DMA/SBUF traffic). PSUM accumulation stays fp32.
  * r/z gates: W_hh and W_ih matmuls accumulate into the same PSUM,
    sigmoid applied straight out of PSUM with fused bias.
  * LayerNorm over H (partition dim) via ones-vector matmul reductions,
    batched over blocks of 4 steps; mean/rstd computed fp32, cast to
    bf16 and broadcast back across partitions with a DRAM-bounce DMA.
  * h_exp is h_last broadcast over L: computed on device as h_last,
    replicated during the host-side unshard.
"""
import sys

sys.path.insert(0, "/opt/trn_rl_repo")

import numpy as np
import ml_dtypes

import concourse.bass as bass
import concourse.tile as tile
from concourse import bacc, mybir
from concourse.bass_utils import run_bass_kernel_spmd

F32 = mybir.dt.float32
BF = mybir.dt.bfloat16
AF = mybir.ActivationFunctionType
ALU = mybir.AluOpType

N, L, H = 128, 512, 512
NCORES = 8
NB = N // NCORES          # batch rows per core = 16
C = 16                    # chunks per sequence
KS = L // C               # main steps per chunk = 32
S = NB * C                # columns per core = 256
HT = H // 128             # h partition tiles = 4
GT = 3 * H // 128         # gate partition tiles = 12
BLK = 4                   # LN block (main steps)
NBLK = KS // BLK          # 8

BF_NP = ml_dtypes.bfloat16


def _bcast_ap(row_ap, parts=128):
    """DRAM row AP -> partition-broadcast AP (step 0 over partitions)."""
    return bass.AP(
        tensor=row_ap.tensor,
        offset=row_ap.offset,
        ap=[[0, parts]] + [list(d) for d in row_ap.ap],
    )


def build_program(R=16, triv_gb=False):
    T = R + KS
    nc = bacc.Bacc("TRN2", target_bir_lowering=False)

    xs_d = nc.declare_dram_parameter("xs", [HT, 128, T, S], BF, isOutput=False)
    ms_d = nc.declare_dram_parameter("ms", [T, S], BF, isOutput=False)
    h0m_d = nc.declare_dram_parameter("h0m", [HT, 128, NB], BF, isOutput=False)
    wih_d = nc.declare_dram_parameter("wih", [HT, 128, 3 * H], BF, isOutput=False)
    whh_d = nc.declare_dram_parameter("whh", [HT, 128, 3 * H], BF, isOutput=False)
    brz_d = nc.declare_dram_parameter("brz", [128, 8], F32, isOutput=False)
    bhn_d = nc.declare_dram_parameter("bhn", [128, HT], F32, isOutput=False)
    bin_d = nc.declare_dram_parameter("bin", [128, HT], F32, isOutput=False)
    gam_d = nc.declare_dram_parameter("gam", [128, HT], F32, isOutput=False)
    bet_d = nc.declare_dram_parameter("bet", [128, HT], F32, isOutput=False)
    ones_d = nc.declare_dram_parameter("ones", [128, 1], BF, isOutput=False)
    zeros_d = nc.declare_dram_parameter("zeros", [128, S], BF, isOutput=False)

    out_d = nc.declare_dram_parameter("out_st", [HT, 128, KS, S], BF, isOutput=True)
    hl_d = nc.declare_dram_parameter("hlast", [HT, 128, NB], BF, isOutput=True)

    scr = nc.dram_tensor("lnscr", [NBLK, 2048], BF)

    with tile.TileContext(nc) as tc:
        with (
            tc.tile_pool(name="const", bufs=1) as cst,
            tc.tile_pool(name="sb", bufs=1) as sb,
            tc.tile_pool(name="rp", bufs=4, space="PSUM") as rp,
            tc.tile_pool(name="ip", bufs=4, space="PSUM") as ip,
        ):
            # ---- constants (wih first: needed by the t=0 prefill) ----
            wih_sb, whh_sb, h0m_sb = [], [], []
            for k in range(HT):
                w1 = cst.tile([128, 3 * H], BF, name=f"wih_sb{k}", tag=f"wih{k}")
                nc.sync.dma_start(out=w1, in_=wih_d[k, :, :])
                wih_sb.append(w1)
            x0 = []
            for k in range(HT):
                x1 = sb.tile([128, S], BF, name=f"xt0_{k}", tag="xt", bufs=8)
                nc.sync.dma_start(out=x1, in_=xs_d[k, :, 0, :])
                x0.append(x1)
            for k in range(HT):
                w2 = cst.tile([128, 3 * H], BF, name=f"whh_sb{k}", tag=f"whh{k}")
                nc.sync.dma_start(out=w2, in_=whh_d[k, :, :])
                whh_sb.append(w2)
                hm = cst.tile([128, NB], BF, name=f"h0m_sb{k}", tag=f"h0m{k}")
                nc.sync.dma_start(out=hm, in_=h0m_d[k, :, :])
                h0m_sb.append(hm)
            brz_sb = cst.tile([128, 8], F32, name="brz_sb", tag="brz")
            nc.sync.dma_start(out=brz_sb, in_=brz_d[:, :])
            bhn_sb = cst.tile([128, HT], F32, name="bhn_sb", tag="bhn")
            nc.sync.dma_start(out=bhn_sb, in_=bhn_d[:, :])
            bin_sb = cst.tile([128, HT], F32, name="bin_sb", tag="bin")
            nc.sync.dma_start(out=bin_sb, in_=bin_d[:, :])
            gam_sb = cst.tile([128, HT], F32, name="gam_sb", tag="gam")
            nc.sync.dma_start(out=gam_sb, in_=gam_d[:, :])
            bet_sb = cst.tile([128, HT], F32, name="bet_sb", tag="bet")
            nc.sync.dma_start(out=bet_sb, in_=bet_d[:, :])
            ones_sb = cst.tile([128, 1], BF, name="ones_sb", tag="ones")
            nc.sync.dma_start(out=ones_sb, in_=ones_d[:, :])
            eps_sb = cst.tile([1, 1], F32, name="eps_sb", tag="eps")
            nc.vector.memset(eps_sb, 1e-5)

            # ---- initial (zero) state ----
            s_cur = []
            for k in range(HT):
                st = sb.tile([128, S], BF, name=f"s_init{k}", tag="state", bufs=8)
                nc.sync.dma_start(out=st, in_=zeros_d[:, :])
                s_cur.append(st)

            out_flat = [out_d[k, :, :, :].rearrange("p t s -> p (t s)") for k in range(HT)]

            def load_x(t):
                xt = []
                for k in range(HT):
                    x1 = sb.tile([128, S], BF, name=f"xt{t}_{k}", tag="xt", bufs=8)
                    nc.sync.dma_start(out=x1, in_=xs_d[k, :, t, :])
                    xt.append(x1)
                return xt

            def prefill_gxn_mm(t, xt):
                # complete psum groups for gx_n of step t (wih only)
                gx_ps = [
                    ip.tile([128, 512], F32, name=f"gx{t}_{j}", tag="ip")
                    for j in range(2)
                ]
                for k4 in range(4):
                    j = 8 + k4
                    oap = gx_ps[k4 // 2][:, (k4 % 2) * 256 : (k4 % 2) * 256 + 256]
                    for k in range(HT):
                        nc.tensor.matmul(
                            oap, wih_sb[k][:, j * 128 : (j + 1) * 128], xt[k],
                            start=(k == 0), stop=(k == HT - 1))
                return gx_ps

            def drain_gxn(t, gx_ps):
                # drain gx_n to SBUF on ACT with b_in folded in (emitted
                # after the current step's tanh so ACT stays in chain order)
                gxs = []
                for k in range(HT):
                    g1 = sb.tile([128, S], BF, name=f"gxs{t}_{k}", tag="gxs", bufs=4)
                    nc.scalar.activation(
                        out=g1,
                        in_=gx_ps[k // 2][:, (k % 2) * 256 : (k % 2) * 256 + 256],
                        func=AF.Identity, bias=bin_sb[:, k : k + 1], scale=1.0)
                    gxs.append(g1)
                return gxs

            def prefill_r(t, xt):
                # open accumulation groups for the r gate of step t
                # (one full PSUM bank per r g-tile; closed by whh next step)
                r_ps = [
                    rp.tile([128, 256], F32, name=f"r{t}_{j}", tag="rp")
                    for j in range(4)
                ]
                for j in range(4):
                    for k in range(HT):
                        nc.tensor.matmul(
                            r_ps[j], wih_sb[k][:, j * 128 : (j + 1) * 128], xt[k],
                            start=(k == 0), stop=False)
                return r_ps

            xt = x0
            gx_ps0 = prefill_gxn_mm(0, xt)
            gxs_cur = drain_gxn(0, gx_ps0)
            r_ps = prefill_r(0, xt)

            # deferred LN work: step index -> [thunk, ...]
            pending = {}

            def defer(step, fn):
                pending.setdefault(step, []).append(fn)

            y_blk = None
            for t in range(T):
                main = t >= R
                toff = (t - R) % BLK
                blk = (t - R) // BLK

                if t + 1 < T:
                    xt_nxt = load_x(t + 1)
                    mk = sb.tile([128, S], BF, name=f"mk{t}", tag="mask", bufs=3)
                    nc.scalar.dma_start(out=mk, in_=_bcast_ap(ms_d[t + 1, :]))

                # -- close r groups with the recurrent part --
                # k-outer: the first matmuls need only s_cur[0], so PE can
                # start as soon as the first state tile is masked
                for k in range(HT):
                    for j in range(4):
                        nc.tensor.matmul(
                            r_ps[j], whh_sb[k][:, j * 128 : (j + 1) * 128], s_cur[k],
                            start=False, stop=(k == HT - 1))
                # -- gh_n (whh only, complete groups) --
                gh_ps = [
                    ip.tile([128, 512], F32, name=f"gh{t}_{j}", tag="ip")
                    for j in range(2)
                ]
                for k4 in range(4):
                    j = 8 + k4
                    oap = gh_ps[k4 // 2][:, (k4 % 2) * 256 : (k4 % 2) * 256 + 256]
                    for k in range(HT):
                        nc.tensor.matmul(
                            oap, whh_sb[k][:, j * 128 : (j + 1) * 128], s_cur[k],
                            start=(k == 0), stop=(k == HT - 1))
                # -- z gate (whh + wih complete groups, in-step) --
                z_ps = [
                    ip.tile([128, 512], F32, name=f"z{t}_{j}", tag="ip")
                    for j in range(2)
                ]
                for j4 in range(4):
                    j = 4 + j4
                    oap = z_ps[j4 // 2][:, (j4 % 2) * 256 : (j4 % 2) * 256 + 256]
                    for k in range(HT):
                        nc.tensor.matmul(
                            oap, whh_sb[k][:, j * 128 : (j + 1) * 128], s_cur[k],
                            start=(k == 0), stop=False)
                    for k in range(HT):
                        nc.tensor.matmul(
                            oap, wih_sb[k][:, j * 128 : (j + 1) * 128], xt[k],
                            start=False, stop=(k == HT - 1))

                # -- prefill next step (PE stays busy during gate math) --
                if t + 1 < T:
                    gx_ps_nxt = prefill_gxn_mm(t + 1, xt_nxt)
                    r_nxt = prefill_r(t + 1, xt_nxt)

                # -- sigmoids straight from PSUM (bias = b_ih + b_hh) --
                r_t, z_t = [], []
                for k in range(HT):
                    rt = sb.tile([128, S], BF, name=f"rt{t}_{k}", tag="rt", bufs=6)
                    nc.scalar.activation(
                        out=rt, in_=r_ps[k],
                        func=AF.Sigmoid, bias=brz_sb[:, k : k + 1], scale=1.0)
                    r_t.append(rt)
                for k in range(HT):
                    j = 4 + k
                    zt = sb.tile([128, S], BF, name=f"zt{t}_{k}", tag="zt", bufs=6)
                    nc.scalar.activation(
                        out=zt, in_=z_ps[k // 2][:, (k % 2) * 256 : (k % 2) * 256 + 256],
                        func=AF.Sigmoid, bias=brz_sb[:, j : j + 1], scale=1.0)
                    z_t.append(zt)
                # -- n gate --
                gxs = gxs_cur
                n_t = []
                for k in range(HT):
                    stt = sb.tile([128, S], BF, name=f"st{t}_{k}", tag="stt", bufs=4)
                    nc.vector.scalar_tensor_tensor(
                        out=stt,
                        in0=gh_ps[k // 2][:, (k % 2) * 256 : (k % 2) * 256 + 256],
                        scalar=bhn_sb[:, k : k + 1], in1=r_t[k],
                        op0=ALU.add, op1=ALU.mult)
                    u = sb.tile([128, S], BF, name=f"u{t}_{k}", tag="u", bufs=6)
                    nc.vector.tensor_add(u, stt, gxs[k])
                    nt = sb.tile([128, S], BF, name=f"nt{t}_{k}", tag="nt", bufs=6)
                    nc.scalar.activation(
                        out=nt, in_=u, func=AF.Tanh, scale=1.0)
                    n_t.append(nt)
                # next step's gx_n drain goes on ACT *after* this step's
                # chain ops so it cannot head-of-line block them
                if t + 1 < T:
                    gxs_nxt = drain_gxn(t + 1, gx_ps_nxt)
                # -- hidden update: hn = (s - n)*z + n --
                hn = []
                for k in range(HT):
                    # whole update on DVE: Pool is ~3x slower per op and this
                    # is the recurrent critical path
                    t1 = sb.tile([128, S], BF, name=f"t1{t}_{k}", tag="t1", bufs=4)
                    nc.vector.tensor_sub(t1, s_cur[k], n_t[k])
                    t2 = sb.tile([128, S], BF, name=f"t2{t}_{k}", tag="t2", bufs=4)
                    nc.vector.tensor_mul(t2, t1, z_t[k])
                    hh = sb.tile([128, S], BF, name=f"hn{t}_{k}", tag="hn", bufs=8)
                    nc.vector.tensor_add(hh, t2, n_t[k])
                    hn.append(hh)

                # -- residual into LN block buffer --
                if main:
                    if toff == 0:
                        y_blk = [
                            sb.tile([128, BLK * S], BF, name=f"yb{blk}_{k}",
                                    tag=f"yb{k}", bufs=4)
                            for k in range(HT)
                        ]
                        y2 = [
                            sb.tile([128, BLK * S], BF, name=f"y2_{blk}_{k}",
                                    tag=f"y2_{k}", bufs=2)
                            for k in range(HT)
                        ]
                    for k in range(HT):
                        # residual + square are off the critical path -> Pool
                        ysl = y_blk[k][:, toff * S : (toff + 1) * S]
                        nc.gpsimd.tensor_add(ysl, hn[k], xt[k])
                        nc.gpsimd.tensor_mul(
                            y2[k][:, toff * S : (toff + 1) * S], ysl, ysl)

                # -- next state (masked), h0 injection at entry to main --
                if t + 1 < T:
                    s_nxt = []
                    for k in range(HT):
                        sn = sb.tile([128, S], BF, name=f"s{t + 1}_{k}",
                                     tag="state", bufs=8)
                        nc.vector.tensor_mul(sn, hn[k], mk)
                        s_nxt.append(sn)
                    if t + 1 == R:
                        for k in range(HT):
                            nc.vector.tensor_copy(
                                s_nxt[k][:, 0:S:C], h0m_sb[k])
                    s_cur = s_nxt
                    xt = xt_nxt
                    gxs_cur = gxs_nxt
                    r_ps = r_nxt

                # -- LayerNorm for a finished block: stats matmuls + PSUM
                # drains now; the rest of the chain is deferred across the
                # following steps so the recurrent chain never waits on it --
                if main and toff == BLK - 1:
                    FB = BLK * S  # 1024
                    mu_ps = [
                        ip.tile([1, 512], F32, name=f"mu{blk}_{h}", tag="ip")
                        for h in range(2)
                    ]
                    for half in range(2):
                        for k in range(HT):
                            nc.tensor.matmul(
                                mu_ps[half], ones_sb,
                                y_blk[k][:, half * 512 : (half + 1) * 512],
                                start=(k == 0), stop=(k == HT - 1))
                    mu_s = sb.tile([1, FB], F32, name=f"mus{blk}", tag="mus", bufs=2)
                    for half in range(2):
                        nc.vector.tensor_scalar_mul(
                            mu_s[:, half * 512 : (half + 1) * 512], mu_ps[half],
                            1.0 / H)

                    def ln_ss_var(b, mu_s, y2l):
                        # one step later: y2 (Pool) has had a full step of
                        # slack, so the ss matmuls never wait on it
                        ss_ps = [
                            ip.tile([1, 512], F32, name=f"ss{b}_{h}", tag="ip")
                            for h in range(2)
                        ]
                        for half in range(2):
                            for k in range(HT):
                                nc.tensor.matmul(
                                    ss_ps[half], ones_sb,
                                    y2l[k][:, half * 512 : (half + 1) * 512],
                                    start=(k == 0), stop=(k == HT - 1))
                        ss_s = sb.tile([1, FB], F32, name=f"sss{b}", tag="sss", bufs=2)
                        for half in range(2):
                            nc.scalar.activation(
                                out=ss_s[:, half * 512 : (half + 1) * 512],
                                in_=ss_ps[half], func=AF.Identity, scale=1.0 / H)
                        var_s = sb.tile([1, FB], F32, name=f"var{b}", tag="vars", bufs=2)
                        nc.vector.scalar_tensor_tensor(
                            out=var_s, in0=mu_s, scalar=-1.0, in1=mu_s,
                            op0=ALU.mult, op1=ALU.mult)
                        nc.vector.tensor_add(var_s, var_s, ss_s)
                        return var_s

                    def ln_rstd(b, var_s):
                        std_s = sb.tile([1, FB], F32, name=f"std{b}", tag="stds", bufs=2)
                        nc.scalar.activation(
                            out=std_s, in_=var_s, func=AF.Sqrt, bias=eps_sb, scale=1.0)
                        rst_s = sb.tile([1, FB], F32, name=f"rst{b}", tag="rsts", bufs=2)
                        nc.vector.reciprocal_approx_fast(out=rst_s, in_=std_s)
                        return rst_s

                    def ln_cast(b, mu_s, rst_s):
                        # casts on Pool so neither chain engine is touched
                        mu16 = sb.tile([1, FB], BF, name=f"mu16_{b}", tag="mu16", bufs=2)
                        rst16 = sb.tile([1, FB], BF, name=f"rst16_{b}", tag="rst16", bufs=2)
                        nc.gpsimd.tensor_copy(mu16, mu_s)
                        nc.gpsimd.tensor_copy(rst16, rst_s)
                        nc.gpsimd.dma_start(out=scr[b : b + 1, 0:1024], in_=mu16)
                        nc.gpsimd.dma_start(out=scr[b : b + 1, 1024:2048], in_=rst16)

                    def ln_bcast(b):
                        mu_bc = sb.tile([128, FB], BF, name=f"mubc{b}", tag="mubc", bufs=2)
                        rs_bc = sb.tile([128, FB], BF, name=f"rsbc{b}", tag="rsbc", bufs=2)
                        nc.gpsimd.dma_start(out=mu_bc, in_=_bcast_ap(scr[b, 0:1024]))
                        nc.gpsimd.dma_start(out=rs_bc, in_=_bcast_ap(scr[b, 1024:2048]))
                        return mu_bc, rs_bc

                    def ln_norm_k(b, k, ybk, mu_bc, rs_bc):
                        yn = sb.tile([128, FB], BF, name=f"yn{b}_{k}", tag="yn", bufs=2)
                        nc.gpsimd.tensor_sub(yn, ybk, mu_bc)
                        nc.vector.tensor_mul(yn, yn, rs_bc)
                        if not triv_gb:
                            nc.vector.tensor_scalar(
                                out=yn, in0=yn,
                                scalar1=gam_sb[:, k : k + 1],
                                scalar2=bet_sb[:, k : k + 1],
                                op0=ALU.mult, op1=ALU.add)
                        nc.sync.dma_start(
                            out=out_flat[k][:, b * FB : (b + 1) * FB], in_=yn)

                    def sched(b=blk, mu_s=mu_s, yb=y_blk, y2l=y2):
                        box = {}

                        def s1():
                            box["var"] = ln_ss_var(b, mu_s, y2l)

                        def s2():
                            box["rst"] = ln_rstd(b, box["var"])

                        def s2b():
                            ln_cast(b, mu_s, box["rst"])

                        def s3():
                            box["bc"] = ln_bcast(b)

                        defer(t + 1, s1)
                        defer(t + 2, s2)
                        defer(t + 3, s2b)
                        defer(t + 4, s3)
                        for k in range(HT):
                            defer(t + 5 + k,
                                  lambda k=k: ln_norm_k(b, k, yb[k], *box["bc"]))

                    sched()

                # -- final hidden state (chunk C-1 columns) --
                if t == T - 1:
                    for k in range(HT):
                        nc.sync.dma_start(
                            out=hl_d[k, :, :], in_=hn[k][:, C - 1 : S : C])

                # -- deferred LN work scheduled for this step --
                for fn in pending.pop(t, []):
                    fn()

            # -- flush LN work scheduled past the last step --
            for step in sorted(pending):
                for fn in pending[step]:
                    fn()
            pending.clear()
    nc.compile()
    return nc


def stage_inputs(input, h, is_initial, W_ih, W_hh, b_ih, b_hh, gamma, beta, R):
    """Host-side sharding/staging. Returns per-core input maps."""
    T = R + KS
    x = np.asarray(input, np.float32)
    h0 = np.asarray(h, np.float32)
    ii = np.asarray(is_initial).reshape(N, L)
    W_ih = np.asarray(W_ih, np.float32)
    W_hh = np.asarray(W_hh, np.float32)
    b_ih = np.asarray(b_ih, np.float32)
    b_hh = np.asarray(b_hh, np.float32)
    gamma = np.asarray(gamma, np.float32)
    beta = np.asarray(beta, np.float32)

    mask = 1.0 - ii.astype(np.float32)  # [N, L]

    # l index per (c, t): warm-up reads the R steps before the chunk;
    # chunk 0's warm-up reads l in [KS-R, KS) (discarded garbage).
    l_for = np.empty((C, T), np.int64)
    for c in range(C):
        for t in range(T):
            l = c * KS + (t - R)
            l_for[c, t] = l if l >= 0 else l + KS
    # masks per phase step t: state entering step t is h * ms[t]
    # ms[t, s] with s = n*C + c uses mask at l_for[c, t]
    # (for t=0 it is unused; state starts at zero)

    # weight layouts: wih[k, p, g] = W_ih[g, k*128+p]
    wihT = np.ascontiguousarray(
        W_ih.T.reshape(HT, 128, 3 * H)).astype(BF_NP)
    whhT = np.ascontiguousarray(
        W_hh.T.reshape(HT, 128, 3 * H)).astype(BF_NP)
    brz = (b_ih + b_hh)[: 2 * H].reshape(8, 128).T.copy()        # [128, 8]
    bhn = b_hh[2 * H :].reshape(HT, 128).T.copy()                # [128, 4]
    binn = b_ih[2 * H :].reshape(HT, 128).T.copy()
    gam = gamma.reshape(HT, 128).T.copy()
    bet = beta.reshape(HT, 128).T.copy()
    ones = np.ones((128, 1), BF_NP)
    zeros = np.zeros((128, S), BF_NP)

    in_maps = []
    for core in range(NCORES):
        n0 = core * NB
        xc = x[n0 : n0 + NB]              # [NB, L, H]
        # xs[k, p, t, s] = x[n, l_for[c, t], k*128+p], s = n*C + c
        xg = xc[:, l_for, :]              # [NB, C, T, H]
        xs = np.ascontiguousarray(
            xg.transpose(3, 2, 0, 1).reshape(HT, 128, T, S)).astype(BF_NP)
        mg = mask[n0 : n0 + NB][:, l_for]  # [NB, C, T]
        ms = np.ascontiguousarray(
            mg.transpose(2, 0, 1).reshape(T, S)).astype(BF_NP)
        m0 = mask[n0 : n0 + NB, 0]         # [NB]
        h0m = np.ascontiguousarray(
            (h0[n0 : n0 + NB] * m0[:, None]).T.reshape(HT, 128, NB)).astype(BF_NP)
        in_maps.append({
            "xs": xs, "ms": ms, "h0m": h0m,
            "wih": wihT, "whh": whhT, "brz": brz, "bhn": bhn, "bin": binn,
            "gam": gam, "bet": bet, "ones": ones, "zeros": zeros,
        })
    return in_maps


def required_warmup(is_initial):
    """Max distance from a chunk boundary back to the latest reset."""
    ii = np.asarray(is_initial).reshape(N, L)
    need = 0
    for c in range(1, C):
        start = c * KS
        sub = ii[:, :start]
        for n in range(N):
            nz = np.nonzero(sub[n])[0]
            gap = start - nz[-1] if len(nz) else start
            need = max(need, gap)
    return need


def unstage_outputs(results):
    out = np.empty((N, L, H), np.float32)
    h_last = np.empty((N, H), np.float32)
    for core in range(NCORES):
        n0 = core * NB
        st = np.asarray(results[core]["out_st"], dtype=np.float32)  # [HT,128,KS,S]
        o = st.reshape(HT, 128, KS, NB, C).transpose(3, 4, 2, 0, 1)
        out[n0 : n0 + NB] = o.reshape(NB, L, H)
        hl = np.asarray(results[core]["hlast"], dtype=np.float32)   # [HT,128,NB]
        h_last[n0 : n0 + NB] = hl.transpose(2, 0, 1).reshape(NB, H)
    h_exp = np.broadcast_to(h_last[:, None, :], (N, L, H)).copy()
    return out, h_exp


_PROGRAM_CACHE = {}


def kernel(input, h, is_initial, W_ih, W_hh, b_ih, b_hh, gamma, beta):
    need = required_warmup(is_initial)
    R = max(4, need)
    triv = bool(
        np.all(np.asarray(gamma) == 1.0) and np.all(np.asarray(beta) == 0.0))
    key = (R, triv)
    if key not in _PROGRAM_CACHE:
        _PROGRAM_CACHE[key] = build_program(R, triv_gb=triv)
    nc = _PROGRAM_CACHE[key]
    in_maps = stage_inputs(
        input, h, is_initial, W_ih, W_hh, b_ih, b_hh, gamma, beta, R)
    res = run_bass_kernel_spmd(nc, in_maps, list(range(NCORES))).results
    return unstage_outputs(res)


# revision 20
# speedup vs baseline: 1.0549x; 1.0549x over previous
"""Trainium2 Bass kernel for masked-GRU + residual + LayerNorm.

Problem: N=128 sequences of length L=512, hidden H=512.
  gx = x @ W_ih.T + b_ih            (precomputable input projection)
  per step l: hc = h * (1-is_initial[l]); gh = hc @ W_hh.T + b_hh
    r = sig(gx_r+gh_r); z = sig(gx_z+gh_z); n = tanh(gx_n + r*gh_n)
    h = (1-z)*n + z*hc
  out = LayerNorm(seq + x) * gamma + beta;  h_exp = broadcast(h_last)

Strategy:
  * Data parallel: 16 batch rows per core (8 cores).
  * Sequence-chunk parallel: each L=512 sequence is split into C=16
    chunks of 32 steps. Chunks are processed as independent columns,
    made exact by an R-step warm-up: the state entering chunk c only
    depends on inputs back to the latest reset (is_initial==1), and the
    data has a reset within every 16-step window (verified at runtime;
    R grows if needed). Chunk 0 injects the true h0 at its first step.
  * Per-core per-step state: hT [H=512 (4 partition tiles), 256 cols].
    Weights stationary, moving free dim 256, all matmul operands bf16
    (1 cycle/row on PE, same as f32r, but 2x DVE elementwise and half
    the DMA/SBUF traffic). PSUM accumulation stays fp32.
  * r/z gates: W_hh and W_ih matmuls accumulate into the same PSUM,
    sigmoid applied straight out of PSUM with fused bias.
  * LayerNorm over H (partition dim) via ones-vector matmul reductions,
    batched over blocks of 4 steps; mean/rstd computed fp32, cast to
    bf16 and broadcast back across partitions with a DRAM-bounce DMA.
  * h_exp is h_last broadcast over L: computed on device as h_last,
    replicated during the host-side unshard.
"""
import sys

sys.path.insert(0, "/opt/trn_rl_repo")

import numpy as np
import ml_dtypes

import concourse.bass as bass
import concourse.tile as tile
from concourse import bacc, mybir
from concourse.bass_utils import run_bass_kernel_spmd

F32 = mybir.dt.float32
BF = mybir.dt.bfloat16
AF = mybir.ActivationFunctionType
ALU = mybir.AluOpType

N, L, H = 128, 512, 512
NCORES = 8
NB = N // NCORES          # batch rows per core = 16
C = 16                    # chunks per sequence
KS = L // C               # main steps per chunk = 32
S = NB * C                # columns per core = 256
HT = H // 128             # h partition tiles = 4
GT = 3 * H // 128         # gate partition tiles = 12
BLK = 4                   # LN block (main steps)
NBLK = KS // BLK          # 8

BF_NP = ml_dtypes.bfloat16


def _bcast_ap(row_ap, parts=128):
    """DRAM row AP -> partition-broadcast AP (step 0 over partitions)."""
    return bass.AP(
        tensor=row_ap.tensor,
        offset=row_ap.offset,
        ap=[[0, parts]] + [list(d) for d in row_ap.ap],
    )


def build_program(R=16, triv_gb=False):
    T = R + KS
    nc = bacc.Bacc("TRN2", target_bir_lowering=False)

    xs_d = nc.declare_dram_parameter("xs", [HT, 128, T, S], BF, isOutput=False)
    ms_d = nc.declare_dram_parameter("ms", [T, S], BF, isOutput=False)
    h0m_d = nc.declare_dram_parameter("h0m", [HT, 128, NB], BF, isOutput=False)
    wih_d = nc.declare_dram_parameter("wih", [HT, 128, 3 * H], BF, isOutput=False)
    whh_d = nc.declare_dram_parameter("whh", [HT, 128, 3 * H], BF, isOutput=False)
    brz_d = nc.declare_dram_parameter("brz", [128, 8], F32, isOutput=False)
    bhn_d = nc.declare_dram_parameter("bhn", [128, HT], F32, isOutput=False)
    bin_d = nc.declare_dram_parameter("bin", [128, HT], F32, isOutput=False)
    gam_d = nc.declare_dram_parameter("gam", [128, HT], F32, isOutput=False)
    bet_d = nc.declare_dram_parameter("bet", [128, HT], F32, isOutput=False)
    ones_d = nc.declare_dram_parameter("ones", [128, 1], BF, isOutput=False)
    zeros_d = nc.declare_dram_parameter("zeros", [128, S], BF, isOutput=False)

    out_d = nc.declare_dram_parameter("out_st", [HT, 128, KS, S], BF, isOutput=True)
    hl_d = nc.declare_dram_parameter("hlast", [HT, 128, NB], BF, isOutput=True)

    scr = nc.dram_tensor("lnscr", [NBLK, 2048], BF)

    with tile.TileContext(nc) as tc:
        with (
            tc.tile_pool(name="const", bufs=1) as cst,
            tc.tile_pool(name="sb", bufs=1) as sb,
            tc.tile_pool(name="rp", bufs=4, space="PSUM") as rp,
            tc.tile_pool(name="ip", bufs=4, space="PSUM") as ip,
        ):
            # ---- constants (wih first: needed by the t=0 prefill) ----
            wih_sb, whh_sb, h0m_sb = [], [], []
            for k in range(HT):
                w1 = cst.tile([128, 3 * H], BF, name=f"wih_sb{k}", tag=f"wih{k}")
                nc.sync.dma_start(out=w1, in_=wih_d[k, :, :])
                wih_sb.append(w1)
            x0 = []
            for k in range(HT):
                x1 = sb.tile([128, S], BF, name=f"xt0_{k}", tag="xt", bufs=8)
                nc.sync.dma_start(out=x1, in_=xs_d[k, :, 0, :])
                x0.append(x1)
            for k in range(HT):
                w2 = cst.tile([128, 3 * H], BF, name=f"whh_sb{k}", tag=f"whh{k}")
                nc.sync.dma_start(out=w2, in_=whh_d[k, :, :])
                whh_sb.append(w2)
                hm = cst.tile([128, NB], BF, name=f"h0m_sb{k}", tag=f"h0m{k}")
                nc.sync.dma_start(out=hm, in_=h0m_d[k, :, :])
                h0m_sb.append(hm)
            brz_sb = cst.tile([128, 8], F32, name="brz_sb", tag="brz")
            nc.sync.dma_start(out=brz_sb, in_=brz_d[:, :])
            bhn_sb = cst.tile([128, HT], F32, name="bhn_sb", tag="bhn")
            nc.sync.dma_start(out=bhn_sb, in_=bhn_d[:, :])
            bin_sb = cst.tile([128, HT], F32, name="bin_sb", tag="bin")
            nc.sync.dma_start(out=bin_sb, in_=bin_d[:, :])
            gam_sb = cst.tile([128, HT], F32, name="gam_sb", tag="gam")
            nc.sync.dma_start(out=gam_sb, in_=gam_d[:, :])
            bet_sb = cst.tile([128, HT], F32, name="bet_sb", tag="bet")
            nc.sync.dma_start(out=bet_sb, in_=bet_d[:, :])
            ones_sb = cst.tile([128, 1], BF, name="ones_sb", tag="ones")
            nc.sync.dma_start(out=ones_sb, in_=ones_d[:, :])
            eps_sb = cst.tile([1, 1], F32, name="eps_sb", tag="eps")
            nc.vector.memset(eps_sb, 1e-5)

            # ---- initial (zero) state ----
            s_cur = []
            for k in range(HT):
                st = sb.tile([128, S], BF, name=f"s_init{k}", tag="state", bufs=8)
                nc.sync.dma_start(out=st, in_=zeros_d[:, :])
                s_cur.append(st)

            out_flat = [out_d[k, :, :, :].rearrange("p t s -> p (t s)") for k in range(HT)]

            def load_x(t):
                xt = []
                for k in range(HT):
                    x1 = sb.tile([128, S], BF, name=f"xt{t}_{k}", tag="xt", bufs=8)
                    nc.sync.dma_start(out=x1, in_=xs_d[k, :, t, :])
                    xt.append(x1)
                return xt

            def prefill_gxn_mm(t, xt):
                # complete psum groups for gx_n of step t (wih only)
                gx_ps = [
                    ip.tile([128, 512], F32, name=f"gx{t}_{j}", tag="ip")
                    for j in range(2)
                ]
                for k4 in range(4):
                    j = 8 + k4
                    oap = gx_ps[k4 // 2][:, (k4 % 2) * 256 : (k4 % 2) * 256 + 256]
                    for k in range(HT):
                        nc.tensor.matmul(
                            oap, wih_sb[k][:, j * 128 : (j + 1) * 128], xt[k],
                            start=(k == 0), stop=(k == HT - 1))
                return gx_ps

            def drain_gxn(t, gx_ps):
                # drain gx_n to SBUF on ACT with b_in folded in (emitted
                # after the current step's tanh so ACT stays in chain order)
                gxs = []
                for k in range(HT):
                    g1 = sb.tile([128, S], BF, name=f"gxs{t}_{k}", tag="gxs", bufs=4)
                    nc.scalar.activation(
                        out=g1,
                        in_=gx_ps[k // 2][:, (k % 2) * 256 : (k % 2) * 256 + 256],
                        func=AF.Identity, bias=bin_sb[:, k : k + 1], scale=1.0)
                    gxs.append(g1)
                return gxs

            def prefill_r(t, xt):
                # open accumulation groups for the r gate of step t
                # (one full PSUM bank per r g-tile; closed by whh next step)
                r_ps = [
                    rp.tile([128, 256], F32, name=f"r{t}_{j}", tag="rp")
                    for j in range(4)
                ]
                for j in range(4):
                    for k in range(HT):
                        nc.tensor.matmul(
                            r_ps[j], wih_sb[k][:, j * 128 : (j + 1) * 128], xt[k],
                            start=(k == 0), stop=False)
                return r_ps

            xt = x0
            gx_ps0 = prefill_gxn_mm(0, xt)
            gxs_cur = drain_gxn(0, gx_ps0)
            r_ps = prefill_r(0, xt)

            # deferred LN work: step index -> [thunk, ...]
            pending = {}

            def defer(step, fn):
                pending.setdefault(step, []).append(fn)

            y_blk = None
            for t in range(T):
                main = t >= R
                toff = (t - R) % BLK
                blk = (t - R) // BLK

                if t + 1 < T:
                    xt_nxt = load_x(t + 1)
                    mk = sb.tile([128, S], BF, name=f"mk{t}", tag="mask", bufs=3)
                    nc.scalar.dma_start(out=mk, in_=_bcast_ap(ms_d[t + 1, :]))

                # -- close r groups with the recurrent part --
                # k-outer: the first matmuls need only s_cur[0], so PE can
                # start as soon as the first state tile is masked
                for k in range(HT):
                    for j in range(4):
                        nc.tensor.matmul(
                            r_ps[j], whh_sb[k][:, j * 128 : (j + 1) * 128], s_cur[k],
                            start=False, stop=(k == HT - 1))
                # -- gh_n (whh only, complete groups) --
                gh_ps = [
                    ip.tile([128, 512], F32, name=f"gh{t}_{j}", tag="ip")
                    for j in range(2)
                ]
                for k4 in range(4):
                    j = 8 + k4
                    oap = gh_ps[k4 // 2][:, (k4 % 2) * 256 : (k4 % 2) * 256 + 256]
                    for k in range(HT):
                        nc.tensor.matmul(
                            oap, whh_sb[k][:, j * 128 : (j + 1) * 128], s_cur[k],
                            start=(k == 0), stop=(k == HT - 1))
                # -- z gate (whh + wih complete groups, in-step) --
                z_ps = [
                    ip.tile([128, 512], F32, name=f"z{t}_{j}", tag="ip")
                    for j in range(2)
                ]
                for j4 in range(4):
                    j = 4 + j4
                    oap = z_ps[j4 // 2][:, (j4 % 2) * 256 : (j4 % 2) * 256 + 256]
                    for k in range(HT):
                        nc.tensor.matmul(
                            oap, whh_sb[k][:, j * 128 : (j + 1) * 128], s_cur[k],
                            start=(k == 0), stop=False)
                    for k in range(HT):
                        nc.tensor.matmul(
                            oap, wih_sb[k][:, j * 128 : (j + 1) * 128], xt[k],
                            start=False, stop=(k == HT - 1))

                # -- prefill next step (PE stays busy during gate math) --
                if t + 1 < T:
                    gx_ps_nxt = prefill_gxn_mm(t + 1, xt_nxt)
                    r_nxt = prefill_r(t + 1, xt_nxt)

                # -- sigmoids straight from PSUM (bias = b_ih + b_hh) --
                r_t, z_t = [], []
                for k in range(HT):
                    rt = sb.tile([128, S], BF, name=f"rt{t}_{k}", tag="rt", bufs=6)
                    nc.scalar.activation(
                        out=rt, in_=r_ps[k],
                        func=AF.Sigmoid, bias=brz_sb[:, k : k + 1], scale=1.0)
                    r_t.append(rt)
                for k in range(HT):
                    j = 4 + k
                    zt = sb.tile([128, S], BF, name=f"zt{t}_{k}", tag="zt", bufs=6)
                    nc.scalar.activation(
                        out=zt, in_=z_ps[k // 2][:, (k % 2) * 256 : (k % 2) * 256 + 256],
                        func=AF.Sigmoid, bias=brz_sb[:, j : j + 1], scale=1.0)
                    z_t.append(zt)
                # -- n gate --
                gxs = gxs_cur
                n_t = []
                for k in range(HT):
                    stt = sb.tile([128, S], BF, name=f"st{t}_{k}", tag="stt", bufs=4)
                    nc.vector.scalar_tensor_tensor(
                        out=stt,
                        in0=gh_ps[k // 2][:, (k % 2) * 256 : (k % 2) * 256 + 256],
                        scalar=bhn_sb[:, k : k + 1], in1=r_t[k],
                        op0=ALU.add, op1=ALU.mult)
                    u = sb.tile([128, S], BF, name=f"u{t}_{k}", tag="u", bufs=6)
                    nc.vector.tensor_add(u, stt, gxs[k])
                    nt = sb.tile([128, S], BF, name=f"nt{t}_{k}", tag="nt", bufs=6)
                    nc.scalar.activation(
                        out=nt, in_=u, func=AF.Tanh, scale=1.0)
                    n_t.append(nt)
                # next step's gx_n drain goes on ACT *after* this step's
                # chain ops so it cannot head-of-line block them
                if t + 1 < T:
                    gxs_nxt = drain_gxn(t + 1, gx_ps_nxt)
                # -- hidden update: hn = (s - n)*z + n --
                hn = []
                for k in range(HT):
                    # t1 on Pool runs concurrent with the DVE t2/hh stream
                    t1 = sb.tile([128, S], BF, name=f"t1{t}_{k}", tag="t1", bufs=4)
                    nc.gpsimd.tensor_sub(t1, s_cur[k], n_t[k])
                    t2 = sb.tile([128, S], BF, name=f"t2{t}_{k}", tag="t2", bufs=4)
                    nc.vector.tensor_mul(t2, t1, z_t[k])
                    hh = sb.tile([128, S], BF, name=f"hn{t}_{k}", tag="hn", bufs=8)
                    nc.vector.tensor_add(hh, t2, n_t[k])
                    hn.append(hh)

                # -- residual into LN block buffer --
                if main:
                    if toff == 0:
                        y_blk = [
                            sb.tile([128, BLK * S], BF, name=f"yb{blk}_{k}",
                                    tag=f"yb{k}", bufs=4)
                            for k in range(HT)
                        ]
                        y2 = [
                            sb.tile([128, BLK * S], BF, name=f"y2_{blk}_{k}",
                                    tag=f"y2_{k}", bufs=2)
                            for k in range(HT)
                        ]
                    for k in range(HT):
                        ysl = y_blk[k][:, toff * S : (toff + 1) * S]
                        nc.vector.tensor_add(ysl, hn[k], xt[k])
                        nc.gpsimd.tensor_mul(
                            y2[k][:, toff * S : (toff + 1) * S], ysl, ysl)

                # -- next state (masked), h0 injection at entry to main --
                if t + 1 < T:
                    s_nxt = []
                    for k in range(HT):
                        sn = sb.tile([128, S], BF, name=f"s{t + 1}_{k}",
                                     tag="state", bufs=8)
                        nc.vector.tensor_mul(sn, hn[k], mk)
                        s_nxt.append(sn)
                    if t + 1 == R:
                        for k in range(HT):
                            nc.vector.tensor_copy(
                                s_nxt[k][:, 0:S:C], h0m_sb[k])
                    s_cur = s_nxt
                    xt = xt_nxt
                    gxs_cur = gxs_nxt
                    r_ps = r_nxt

                # -- LayerNorm for a finished block: stats matmuls + PSUM
                # drains now; the rest of the chain is deferred across the
                # following steps so the recurrent chain never waits on it --
                if main and toff == BLK - 1:
                    FB = BLK * S  # 1024
                    mu_ps = [
                        ip.tile([1, 512], F32, name=f"mu{blk}_{h}", tag="ip")
                        for h in range(2)
                    ]
                    for half in range(2):
                        for k in range(HT):
                            nc.tensor.matmul(
                                mu_ps[half], ones_sb,
                                y_blk[k][:, half * 512 : (half + 1) * 512],
                                start=(k == 0), stop=(k == HT - 1))
                    mu_s = sb.tile([1, FB], F32, name=f"mus{blk}", tag="mus", bufs=2)
                    for half in range(2):
                        nc.vector.tensor_scalar_mul(
                            mu_s[:, half * 512 : (half + 1) * 512], mu_ps[half],
                            1.0 / H)

                    def ln_ss_var(b, mu_s, y2l):
                        # one step later: y2 (Pool) has had a full step of
                        # slack, so the ss matmuls never wait on it
                        ss_ps = [
                            ip.tile([1, 512], F32, name=f"ss{b}_{h}", tag="ip")
                            for h in range(2)
                        ]
                        for half in range(2):
                            for k in range(HT):
                                nc.tensor.matmul(
                                    ss_ps[half], ones_sb,
                                    y2l[k][:, half * 512 : (half + 1) * 512],
                                    start=(k == 0), stop=(k == HT - 1))
                        ss_s = sb.tile([1, FB], F32, name=f"sss{b}", tag="sss", bufs=2)
                        for half in range(2):
                            nc.scalar.activation(
                                out=ss_s[:, half * 512 : (half + 1) * 512],
                                in_=ss_ps[half], func=AF.Identity, scale=1.0 / H)
                        var_s = sb.tile([1, FB], F32, name=f"var{b}", tag="vars", bufs=2)
                        nc.vector.scalar_tensor_tensor(
                            out=var_s, in0=mu_s, scalar=-1.0, in1=mu_s,
                            op0=ALU.mult, op1=ALU.mult)
                        nc.vector.tensor_add(var_s, var_s, ss_s)
                        return var_s

                    def ln_rstd(b, var_s):
                        std_s = sb.tile([1, FB], F32, name=f"std{b}", tag="stds", bufs=2)
                        nc.scalar.activation(
                            out=std_s, in_=var_s, func=AF.Sqrt, bias=eps_sb, scale=1.0)
                        rst_s = sb.tile([1, FB], F32, name=f"rst{b}", tag="rsts", bufs=2)
                        nc.vector.reciprocal_approx_fast(out=rst_s, in_=std_s)
                        return rst_s

                    def ln_cast(b, mu_s, rst_s):
                        # casts on Pool so neither chain engine is touched
                        mu16 = sb.tile([1, FB], BF, name=f"mu16_{b}", tag="mu16", bufs=2)
                        rst16 = sb.tile([1, FB], BF, name=f"rst16_{b}", tag="rst16", bufs=2)
                        nc.gpsimd.tensor_copy(mu16, mu_s)
                        nc.gpsimd.tensor_copy(rst16, rst_s)
                        nc.gpsimd.dma_start(out=scr[b : b + 1, 0:1024], in_=mu16)
                        nc.gpsimd.dma_start(out=scr[b : b + 1, 1024:2048], in_=rst16)

                    def ln_bcast(b):
                        mu_bc = sb.tile([128, FB], BF, name=f"mubc{b}", tag="mubc", bufs=2)
                        rs_bc = sb.tile([128, FB], BF, name=f"rsbc{b}", tag="rsbc", bufs=2)
                        nc.gpsimd.dma_start(out=mu_bc, in_=_bcast_ap(scr[b, 0:1024]))
                        nc.gpsimd.dma_start(out=rs_bc, in_=_bcast_ap(scr[b, 1024:2048]))
                        return mu_bc, rs_bc

                    def ln_norm_k(b, k, ybk, mu_bc, rs_bc):
                        yn = sb.tile([128, FB], BF, name=f"yn{b}_{k}", tag="yn", bufs=2)
                        nc.vector.tensor_sub(yn, ybk, mu_bc)
                        nc.vector.tensor_mul(yn, yn, rs_bc)
                        if not triv_gb:
                            nc.vector.tensor_scalar(
                                out=yn, in0=yn,
                                scalar1=gam_sb[:, k : k + 1],
                                scalar2=bet_sb[:, k : k + 1],
                                op0=ALU.mult, op1=ALU.add)
                        nc.sync.dma_start(
                            out=out_flat[k][:, b * FB : (b + 1) * FB], in_=yn)

                    def sched(b=blk, mu_s=mu_s, yb=y_blk, y2l=y2):
                        box = {}

                        def s1():
                            box["var"] = ln_ss_var(b, mu_s, y2l)

                        def s2():
                            box["rst"] = ln_rstd(b, box["var"])

                        def s2b():
                            ln_cast(b, mu_s, box["rst"])

                        def s3():
                            box["bc"] = ln_bcast(b)

                        defer(t + 1, s1)
                        defer(t + 2, s2)
                        defer(t + 3, s2b)
                        defer(t + 4, s3)
                        for k in range(HT):
                            defer(t + 5 + k,
                                  lambda k=k: ln_norm_k(b, k, yb[k], *box["bc"]))

                    sched()

                # -- final hidden state (chunk C-1 columns) --
                if t == T - 1:
                    for k in range(HT):
                        nc.sync.dma_start(
                            out=hl_d[k, :, :], in_=hn[k][:, C - 1 : S : C])

                # -- deferred LN work scheduled for this step --
                for fn in pending.pop(t, []):
                    fn()

            # -- flush LN work scheduled past the last step --
            for step in sorted(pending):
                for fn in pending[step]:
                    fn()
            pending.clear()
    nc.compile()
    return nc


def stage_inputs(input, h, is_initial, W_ih, W_hh, b_ih, b_hh, gamma, beta, R):
    """Host-side sharding/staging. Returns per-core input maps."""
    T = R + KS
    x = np.asarray(input, np.float32)
    h0 = np.asarray(h, np.float32)
    ii = np.asarray(is_initial).reshape(N, L)
    W_ih = np.asarray(W_ih, np.float32)
    W_hh = np.asarray(W_hh, np.float32)
    b_ih = np.asarray(b_ih, np.float32)
    b_hh = np.asarray(b_hh, np.float32)
    gamma = np.asarray(gamma, np.float32)
    beta = np.asarray(beta, np.float32)

    mask = 1.0 - ii.astype(np.float32)  # [N, L]

    # l index per (c, t): warm-up reads the R steps before the chunk;
    # chunk 0's warm-up reads l in [KS-R, KS) (discarded garbage).
    l_for = np.empty((C, T), np.int64)
    for c in range(C):
        for t in range(T):
            l = c * KS + (t - R)
            l_for[c, t] = l if l >= 0 else l + KS
    # masks per phase step t: state entering step t is h * ms[t]
    # ms[t, s] with s = n*C + c uses mask at l_for[c, t]
    # (for t=0 it is unused; state starts at zero)

    # weight layouts: wih[k, p, g] = W_ih[g, k*128+p]
    wihT = np.ascontiguousarray(
        W_ih.T.reshape(HT, 128, 3 * H)).astype(BF_NP)
    whhT = np.ascontiguousarray(
        W_hh.T.reshape(HT, 128, 3 * H)).astype(BF_NP)
    brz = (b_ih + b_hh)[: 2 * H].reshape(8, 128).T.copy()        # [128, 8]
    bhn = b_hh[2 * H :].reshape(HT, 128).T.copy()                # [128, 4]
    binn = b_ih[2 * H :].reshape(HT, 128).T.copy()
    gam = gamma.reshape(HT, 128).T.copy()
    bet = beta.reshape(HT, 128).T.copy()
    ones = np.ones((128, 1), BF_NP)
    zeros = np.zeros((128, S), BF_NP)

    in_maps = []
    for core in range(NCORES):
        n0 = core * NB
        xc = x[n0 : n0 + NB]              # [NB, L, H]
        # xs[k, p, t, s] = x[n, l_for[c, t], k*128+p], s = n*C + c
        xg = xc[:, l_for, :]              # [NB, C, T, H]
        xs = np.ascontiguousarray(
            xg.transpose(3, 2, 0, 1).reshape(HT, 128, T, S)).astype(BF_NP)
        mg = mask[n0 : n0 + NB][:, l_for]  # [NB, C, T]
        ms = np.ascontiguousarray(
            mg.transpose(2, 0, 1).reshape(T, S)).astype(BF_NP)
        m0 = mask[n0 : n0 + NB, 0]         # [NB]
        h0m = np.ascontiguousarray(
            (h0[n0 : n0 + NB] * m0[:, None]).T.reshape(HT, 128, NB)).astype(BF_NP)
        in_maps.append({
            "xs": xs, "ms": ms, "h0m": h0m,
            "wih": wihT, "whh": whhT, "brz": brz, "bhn": bhn, "bin": binn,
            "gam": gam, "bet": bet, "ones": ones, "zeros": zeros,
        })
    return in_maps


def required_warmup(is_initial):
    """Max distance from a chunk boundary back to the latest reset."""
    ii = np.asarray(is_initial).reshape(N, L)
    need = 0
    for c in range(1, C):
        start = c * KS
        sub = ii[:, :start]
        for n in range(N):
            nz = np.nonzero(sub[n])[0]
            gap = start - nz[-1] if len(nz) else start
            need = max(need, gap)
    return need


def unstage_outputs(results):
    out = np.empty((N, L, H), np.float32)
    h_last = np.empty((N, H), np.float32)
    for core in range(NCORES):
        n0 = core * NB
        st = np.asarray(results[core]["out_st"], dtype=np.float32)  # [HT,128,KS,S]
        o = st.reshape(HT, 128, KS, NB, C).transpose(3, 4, 2, 0, 1)
        out[n0 : n0 + NB] = o.reshape(NB, L, H)
        hl = np.asarray(results[core]["hlast"], dtype=np.float32)   # [HT,128,NB]
        h_last[n0 : n0 + NB] = hl.transpose(2, 0, 1).reshape(NB, H)
    h_exp = np.broadcast_to(h_last[:, None, :], (N, L, H)).copy()
    return out, h_exp


_PROGRAM_CACHE = {}


def kernel(input, h, is_initial, W_ih, W_hh, b_ih, b_hh, gamma, beta):
    need = required_warmup(is_initial)
    R = max(4, need)
    triv = bool(
        np.all(np.asarray(gamma) == 1.0) and np.all(np.asarray(beta) == 0.0))
    key = (R, triv)
    if key not in _PROGRAM_CACHE:
        _PROGRAM_CACHE[key] = build_program(R, triv_gb=triv)
    nc = _PROGRAM_CACHE[key]
    in_maps = stage_inputs(
        input, h, is_initial, W_ih, W_hh, b_ih, b_hh, gamma, beta, R)
    res = run_bass_kernel_spmd(nc, in_maps, list(range(NCORES))).results
    return unstage_outputs(res)


# revision 24
# speedup vs baseline: 1.1762x; 1.1151x over previous
"""Trainium2 Bass kernel for masked-GRU + residual + LayerNorm.

Problem: N=128 sequences of length L=512, hidden H=512.
  gx = x @ W_ih.T + b_ih            (precomputable input projection)
  per step l: hc = h * (1-is_initial[l]); gh = hc @ W_hh.T + b_hh
    r = sig(gx_r+gh_r); z = sig(gx_z+gh_z); n = tanh(gx_n + r*gh_n)
    h = (1-z)*n + z*hc
  out = LayerNorm(seq + x) * gamma + beta;  h_exp = broadcast(h_last)

Strategy:
  * Data parallel: 16 batch rows per core (8 cores).
  * Sequence-chunk parallel: each L=512 sequence is split into C=16
    chunks of 32 steps. Chunks are processed as independent columns,
    made exact by an R-step warm-up: the state entering chunk c only
    depends on inputs back to the latest reset (is_initial==1), and the
    data has a reset within every 16-step window (verified at runtime;
    R grows if needed). Chunk 0 injects the true h0 at its first step.
  * Per-core per-step state: hT [H=512 (4 partition tiles), 256 cols].
    Weights stationary, moving free dim 256, all matmul operands bf16
    (1 cycle/row on PE, same as f32r, but 2x DVE elementwise and half
    the DMA/SBUF traffic). PSUM accumulation stays fp32.
  * r/z gates: W_hh and W_ih matmuls accumulate into the same PSUM,
    sigmoid applied straight out of PSUM with fused bias.
  * LayerNorm over H (partition dim) via ones-vector matmul reductions,
    batched over blocks of 4 steps; mean/rstd computed fp32, cast to
    bf16 and broadcast back across partitions with a DRAM-bounce DMA.
  * h_exp is h_last broadcast over L: computed on device as h_last,
    replicated during the host-side unshard.
"""
import sys

sys.path.insert(0, "/opt/trn_rl_repo")

import numpy as np
import ml_dtypes

import concourse.bass as bass
import concourse.tile as tile
from concourse import bacc, mybir
from concourse.bass_utils import run_bass_kernel_spmd

F32 = mybir.dt.float32
BF = mybir.dt.bfloat16
AF = mybir.ActivationFunctionType
ALU = mybir.AluOpType

N, L, H = 128, 512, 512
NCORES = 8
NB = N // NCORES          # batch rows per core = 16
C = 16                    # chunks per sequence
KS = L // C               # main steps per chunk = 32
S = NB * C                # columns per core = 256
HT = H // 128             # h partition tiles = 4
GT = 3 * H // 128         # gate partition tiles = 12
BLK = 4                   # LN block (main steps)
NBLK = KS // BLK          # 8

BF_NP = ml_dtypes.bfloat16


def _bcast_ap(row_ap, parts=128):
    """DRAM row AP -> partition-broadcast AP (step 0 over partitions)."""
    return bass.AP(
        tensor=row_ap.tensor,
        offset=row_ap.offset,
        ap=[[0, parts]] + [list(d) for d in row_ap.ap],
    )


def build_program(R=16, triv_gb=False):
    T = R + KS
    nc = bacc.Bacc("TRN2", target_bir_lowering=False)

    xs_d = nc.declare_dram_parameter("xs", [HT, 128, T, S], BF, isOutput=False)
    ms_d = nc.declare_dram_parameter("ms", [T, S], BF, isOutput=False)
    h0m_d = nc.declare_dram_parameter("h0m", [HT, 128, NB], BF, isOutput=False)
    wih_d = nc.declare_dram_parameter("wih", [HT, 128, 3 * H], BF, isOutput=False)
    whh_d = nc.declare_dram_parameter("whh", [HT, 128, 3 * H], BF, isOutput=False)
    brz_d = nc.declare_dram_parameter("brz", [128, 8], F32, isOutput=False)
    bhn_d = nc.declare_dram_parameter("bhn", [128, HT], F32, isOutput=False)
    bin_d = nc.declare_dram_parameter("bin", [128, HT], F32, isOutput=False)
    gam_d = nc.declare_dram_parameter("gam", [128, HT], F32, isOutput=False)
    bet_d = nc.declare_dram_parameter("bet", [128, HT], F32, isOutput=False)
    ones_d = nc.declare_dram_parameter("ones", [128, 1], BF, isOutput=False)
    zeros_d = nc.declare_dram_parameter("zeros", [128, S], BF, isOutput=False)

    out_d = nc.declare_dram_parameter("out_st", [HT, 128, KS, S], BF, isOutput=True)
    hl_d = nc.declare_dram_parameter("hlast", [HT, 128, NB], BF, isOutput=True)

    scr = nc.dram_tensor("lnscr", [NBLK, 2048], BF)

    with tile.TileContext(nc) as tc:
        with (
            tc.tile_pool(name="const", bufs=1) as cst,
            tc.tile_pool(name="sb", bufs=1) as sb,
            tc.tile_pool(name="rp", bufs=4, space="PSUM") as rp,
            tc.tile_pool(name="ip", bufs=4, space="PSUM") as ip,
        ):
            # ---- constants (wih first: needed by the t=0 prefill). Weight
            # loads split into column chunks so they spread across DMA rings
            # and the first matmul starts sooner. ----
            wih_sb, whh_sb, h0m_sb = [], [], []
            WCK = 4
            for k in range(HT):
                w1 = cst.tile([128, 3 * H], BF, name=f"wih_sb{k}", tag=f"wih{k}")
                for c in range(WCK):
                    cw = 3 * H // WCK
                    nc.sync.dma_start(
                        out=w1[:, c * cw : (c + 1) * cw],
                        in_=wih_d[k, :, c * cw : (c + 1) * cw])
                wih_sb.append(w1)
            x0 = []
            for k in range(HT):
                x1 = sb.tile([128, S], BF, name=f"xt0_{k}", tag="xt", bufs=8)
                nc.sync.dma_start(out=x1, in_=xs_d[k, :, 0, :])
                x0.append(x1)
            # initial zero state early: step 0's in-step matmuls need it
            s_cur = []
            for k in range(HT):
                st = sb.tile([128, S], BF, name=f"s_init{k}", tag="state", bufs=8)
                nc.sync.dma_start(out=st, in_=zeros_d[:, :])
                s_cur.append(st)
            for k in range(HT):
                w2 = cst.tile([128, 3 * H], BF, name=f"whh_sb{k}", tag=f"whh{k}")
                for c in range(WCK):
                    cw = 3 * H // WCK
                    nc.sync.dma_start(
                        out=w2[:, c * cw : (c + 1) * cw],
                        in_=whh_d[k, :, c * cw : (c + 1) * cw])
                whh_sb.append(w2)
                hm = cst.tile([128, NB], BF, name=f"h0m_sb{k}", tag=f"h0m{k}")
                nc.sync.dma_start(out=hm, in_=h0m_d[k, :, :])
                h0m_sb.append(hm)
            brz_sb = cst.tile([128, 8], F32, name="brz_sb", tag="brz")
            nc.sync.dma_start(out=brz_sb, in_=brz_d[:, :])
            bhn_sb = cst.tile([128, HT], F32, name="bhn_sb", tag="bhn")
            nc.sync.dma_start(out=bhn_sb, in_=bhn_d[:, :])
            bin_sb = cst.tile([128, HT], F32, name="bin_sb", tag="bin")
            nc.sync.dma_start(out=bin_sb, in_=bin_d[:, :])
            gam_sb = cst.tile([128, HT], F32, name="gam_sb", tag="gam")
            nc.sync.dma_start(out=gam_sb, in_=gam_d[:, :])
            bet_sb = cst.tile([128, HT], F32, name="bet_sb", tag="bet")
            nc.sync.dma_start(out=bet_sb, in_=bet_d[:, :])
            ones_sb = cst.tile([128, 1], BF, name="ones_sb", tag="ones")
            nc.sync.dma_start(out=ones_sb, in_=ones_d[:, :])
            eps_sb = cst.tile([1, 1], F32, name="eps_sb", tag="eps")
            nc.vector.memset(eps_sb, 1e-5)

            out_flat = [out_d[k, :, :, :].rearrange("p t s -> p (t s)") for k in range(HT)]

            def load_x(t):
                xt = []
                for k in range(HT):
                    x1 = sb.tile([128, S], BF, name=f"xt{t}_{k}", tag="xt", bufs=8)
                    nc.sync.dma_start(out=x1, in_=xs_d[k, :, t, :])
                    xt.append(x1)
                return xt

            def prefill_gxn_mm(t, xt):
                # complete psum groups for gx_n of step t (wih only)
                gx_ps = [
                    ip.tile([128, 512], F32, name=f"gx{t}_{j}", tag="ip")
                    for j in range(2)
                ]
                for k4 in range(4):
                    j = 8 + k4
                    oap = gx_ps[k4 // 2][:, (k4 % 2) * 256 : (k4 % 2) * 256 + 256]
                    for k in range(HT):
                        nc.tensor.matmul(
                            oap, wih_sb[k][:, j * 128 : (j + 1) * 128], xt[k],
                            start=(k == 0), stop=(k == HT - 1))
                return gx_ps

            def drain_gxn(t, gx_ps):
                # drain gx_n to SBUF on ACT with b_in folded in (emitted
                # after the current step's tanh so ACT stays in chain order)
                gxs = []
                for k in range(HT):
                    g1 = sb.tile([128, S], BF, name=f"gxs{t}_{k}", tag="gxs", bufs=4)
                    nc.scalar.activation(
                        out=g1,
                        in_=gx_ps[k // 2][:, (k % 2) * 256 : (k % 2) * 256 + 256],
                        func=AF.Identity, bias=bin_sb[:, k : k + 1], scale=1.0)
                    gxs.append(g1)
                return gxs

            def prefill_r(t, xt):
                # open accumulation groups for the r gate of step t
                # (one full PSUM bank per r g-tile; closed by whh next step)
                r_ps = [
                    rp.tile([128, 256], F32, name=f"r{t}_{j}", tag="rp")
                    for j in range(4)
                ]
                for j in range(4):
                    for k in range(HT):
                        nc.tensor.matmul(
                            r_ps[j], wih_sb[k][:, j * 128 : (j + 1) * 128], xt[k],
                            start=(k == 0), stop=False)
                return r_ps

            xt = x0
            gx_ps0 = prefill_gxn_mm(0, xt)
            gxs_cur = drain_gxn(0, gx_ps0)
            r_ps = prefill_r(0, xt)

            # deferred LN work: step index -> [thunk, ...]
            pending = {}

            def defer(step, fn):
                pending.setdefault(step, []).append(fn)

            y_blk = None
            for t in range(T):
                main = t >= R
                toff = (t - R) % BLK
                blk = (t - R) // BLK

                if t + 1 < T:
                    xt_nxt = load_x(t + 1)
                    mk = sb.tile([128, S], BF, name=f"mk{t}", tag="mask", bufs=3)
                    nc.scalar.dma_start(out=mk, in_=_bcast_ap(ms_d[t + 1, :]))

                # -- close r groups with the recurrent part --
                # k-outer: the first matmuls need only s_cur[0], so PE can
                # start as soon as the first state tile is masked
                for k in range(HT):
                    for j in range(4):
                        nc.tensor.matmul(
                            r_ps[j], whh_sb[k][:, j * 128 : (j + 1) * 128], s_cur[k],
                            start=False, stop=(k == HT - 1))
                # -- gh_n (whh only, complete groups) --
                gh_ps = [
                    ip.tile([128, 512], F32, name=f"gh{t}_{j}", tag="ip")
                    for j in range(2)
                ]
                for k4 in range(4):
                    j = 8 + k4
                    oap = gh_ps[k4 // 2][:, (k4 % 2) * 256 : (k4 % 2) * 256 + 256]
                    for k in range(HT):
                        nc.tensor.matmul(
                            oap, whh_sb[k][:, j * 128 : (j + 1) * 128], s_cur[k],
                            start=(k == 0), stop=(k == HT - 1))
                # -- z gate (whh + wih complete groups, in-step) --
                z_ps = [
                    ip.tile([128, 512], F32, name=f"z{t}_{j}", tag="ip")
                    for j in range(2)
                ]
                for j4 in range(4):
                    j = 4 + j4
                    oap = z_ps[j4 // 2][:, (j4 % 2) * 256 : (j4 % 2) * 256 + 256]
                    for k in range(HT):
                        nc.tensor.matmul(
                            oap, whh_sb[k][:, j * 128 : (j + 1) * 128], s_cur[k],
                            start=(k == 0), stop=False)
                    for k in range(HT):
                        nc.tensor.matmul(
                            oap, wih_sb[k][:, j * 128 : (j + 1) * 128], xt[k],
                            start=False, stop=(k == HT - 1))

                # -- prefill next step (PE stays busy during gate math) --
                if t + 1 < T:
                    gx_ps_nxt = prefill_gxn_mm(t + 1, xt_nxt)
                    r_nxt = prefill_r(t + 1, xt_nxt)

                # -- sigmoids straight from PSUM (bias = b_ih + b_hh) --
                r_t, z_t = [], []
                for k in range(HT):
                    rt = sb.tile([128, S], BF, name=f"rt{t}_{k}", tag="rt", bufs=6)
                    nc.scalar.activation(
                        out=rt, in_=r_ps[k],
                        func=AF.Sigmoid, bias=brz_sb[:, k : k + 1], scale=1.0)
                    r_t.append(rt)
                for k in range(HT):
                    j = 4 + k
                    zt = sb.tile([128, S], BF, name=f"zt{t}_{k}", tag="zt", bufs=6)
                    nc.scalar.activation(
                        out=zt, in_=z_ps[k // 2][:, (k % 2) * 256 : (k % 2) * 256 + 256],
                        func=AF.Sigmoid, bias=brz_sb[:, j : j + 1], scale=1.0)
                    z_t.append(zt)
                # -- n gate --
                gxs = gxs_cur
                n_t = []
                for k in range(HT):
                    stt = sb.tile([128, S], BF, name=f"st{t}_{k}", tag="stt", bufs=4)
                    nc.vector.scalar_tensor_tensor(
                        out=stt,
                        in0=gh_ps[k // 2][:, (k % 2) * 256 : (k % 2) * 256 + 256],
                        scalar=bhn_sb[:, k : k + 1], in1=r_t[k],
                        op0=ALU.add, op1=ALU.mult)
                    u = sb.tile([128, S], BF, name=f"u{t}_{k}", tag="u", bufs=6)
                    nc.vector.tensor_add(u, stt, gxs[k])
                    nt = sb.tile([128, S], BF, name=f"nt{t}_{k}", tag="nt", bufs=6)
                    nc.scalar.activation(
                        out=nt, in_=u, func=AF.Tanh, scale=1.0)
                    n_t.append(nt)
                # next step's gx_n drain goes on ACT *after* this step's
                # chain ops so it cannot head-of-line block them
                if t + 1 < T:
                    gxs_nxt = drain_gxn(t + 1, gx_ps_nxt)
                # -- hidden update: hn = (s - n)*z + n --
                hn = []
                for k in range(HT):
                    # t1 on Pool runs concurrent with the DVE t2/hh stream
                    t1 = sb.tile([128, S], BF, name=f"t1{t}_{k}", tag="t1", bufs=4)
                    nc.gpsimd.tensor_sub(t1, s_cur[k], n_t[k])
                    t2 = sb.tile([128, S], BF, name=f"t2{t}_{k}", tag="t2", bufs=4)
                    nc.vector.tensor_mul(t2, t1, z_t[k])
                    hh = sb.tile([128, S], BF, name=f"hn{t}_{k}", tag="hn", bufs=8)
                    nc.vector.tensor_add(hh, t2, n_t[k])
                    hn.append(hh)

                # -- next state (masked) FIRST: it is the recurrent critical
                # path; the residual/y2 below have a whole block of slack --
                if t + 1 < T:
                    s_nxt = []
                    for k in range(HT):
                        sn = sb.tile([128, S], BF, name=f"s{t + 1}_{k}",
                                     tag="state", bufs=8)
                        nc.vector.tensor_mul(sn, hn[k], mk)
                        s_nxt.append(sn)
                    if t + 1 == R:
                        for k in range(HT):
                            nc.vector.tensor_copy(
                                s_nxt[k][:, 0:S:C], h0m_sb[k])

                # -- residual into LN block buffer --
                if main:
                    if toff == 0:
                        y_blk = [
                            sb.tile([128, BLK * S], BF, name=f"yb{blk}_{k}",
                                    tag=f"yb{k}", bufs=4)
                            for k in range(HT)
                        ]
                        y2 = [
                            sb.tile([128, BLK * S], BF, name=f"y2_{blk}_{k}",
                                    tag=f"y2_{k}", bufs=2)
                            for k in range(HT)
                        ]
                    for k in range(HT):
                        ysl = y_blk[k][:, toff * S : (toff + 1) * S]
                        nc.vector.tensor_add(ysl, hn[k], xt[k])
                        nc.gpsimd.tensor_mul(
                            y2[k][:, toff * S : (toff + 1) * S], ysl, ysl)

                if t + 1 < T:
                    s_cur = s_nxt
                    xt = xt_nxt
                    gxs_cur = gxs_nxt
                    r_ps = r_nxt

                # -- LayerNorm for a finished block: stats matmuls + PSUM
                # drains now; the rest of the chain is deferred across the
                # following steps so the recurrent chain never waits on it --
                if main and toff == BLK - 1:
                    FB = BLK * S  # 1024
                    mu_ps = [
                        ip.tile([1, 512], F32, name=f"mu{blk}_{h}", tag="ip")
                        for h in range(2)
                    ]
                    for half in range(2):
                        for k in range(HT):
                            nc.tensor.matmul(
                                mu_ps[half], ones_sb,
                                y_blk[k][:, half * 512 : (half + 1) * 512],
                                start=(k == 0), stop=(k == HT - 1))
                    mu_s = sb.tile([1, FB], F32, name=f"mus{blk}", tag="mus", bufs=2)
                    for half in range(2):
                        nc.vector.tensor_scalar_mul(
                            mu_s[:, half * 512 : (half + 1) * 512], mu_ps[half],
                            1.0 / H)

                    def ln_ss_var(b, mu_s, y2l):
                        # one step later: y2 (Pool) has had a full step of
                        # slack, so the ss matmuls never wait on it
                        ss_ps = [
                            ip.tile([1, 512], F32, name=f"ss{b}_{h}", tag="ip")
                            for h in range(2)
                        ]
                        for half in range(2):
                            for k in range(HT):
                                nc.tensor.matmul(
                                    ss_ps[half], ones_sb,
                                    y2l[k][:, half * 512 : (half + 1) * 512],
                                    start=(k == 0), stop=(k == HT - 1))
                        ss_s = sb.tile([1, FB], F32, name=f"sss{b}", tag="sss", bufs=2)
                        for half in range(2):
                            nc.scalar.activation(
                                out=ss_s[:, half * 512 : (half + 1) * 512],
                                in_=ss_ps[half], func=AF.Identity, scale=1.0 / H)
                        var_s = sb.tile([1, FB], F32, name=f"var{b}", tag="vars", bufs=2)
                        nc.vector.scalar_tensor_tensor(
                            out=var_s, in0=mu_s, scalar=-1.0, in1=mu_s,
                            op0=ALU.mult, op1=ALU.mult)
                        nc.vector.tensor_add(var_s, var_s, ss_s)
                        return var_s

                    def ln_rstd(b, var_s):
                        std_s = sb.tile([1, FB], F32, name=f"std{b}", tag="stds", bufs=2)
                        nc.scalar.activation(
                            out=std_s, in_=var_s, func=AF.Sqrt, bias=eps_sb, scale=1.0)
                        rst_s = sb.tile([1, FB], F32, name=f"rst{b}", tag="rsts", bufs=2)
                        nc.vector.reciprocal_approx_fast(out=rst_s, in_=std_s)
                        return rst_s

                    def ln_cast(b, mu_s, rst_s):
                        # casts on ACT ([1,N] shapes are terrible on GpSimd:
                        # one partition = one Q7 core)
                        mu16 = sb.tile([1, FB], BF, name=f"mu16_{b}", tag="mu16", bufs=2)
                        rst16 = sb.tile([1, FB], BF, name=f"rst16_{b}", tag="rst16", bufs=2)
                        nc.scalar.activation(out=mu16, in_=mu_s, func=AF.Identity, scale=1.0)
                        nc.scalar.activation(out=rst16, in_=rst_s, func=AF.Identity, scale=1.0)
                        nc.gpsimd.dma_start(out=scr[b : b + 1, 0:1024], in_=mu16)
                        nc.gpsimd.dma_start(out=scr[b : b + 1, 1024:2048], in_=rst16)

                    def ln_bcast(b):
                        mu_bc = sb.tile([128, FB], BF, name=f"mubc{b}", tag="mubc", bufs=2)
                        rs_bc = sb.tile([128, FB], BF, name=f"rsbc{b}", tag="rsbc", bufs=2)
                        nc.gpsimd.dma_start(out=mu_bc, in_=_bcast_ap(scr[b, 0:1024]))
                        nc.gpsimd.dma_start(out=rs_bc, in_=_bcast_ap(scr[b, 1024:2048]))
                        return mu_bc, rs_bc

                    def ln_norm_k(b, k, ybk, mu_bc, rs_bc):
                        yn = sb.tile([128, FB], BF, name=f"yn{b}_{k}", tag="yn", bufs=2)
                        nc.vector.tensor_sub(yn, ybk, mu_bc)
                        nc.vector.tensor_mul(yn, yn, rs_bc)
                        if not triv_gb:
                            nc.vector.tensor_scalar(
                                out=yn, in0=yn,
                                scalar1=gam_sb[:, k : k + 1],
                                scalar2=bet_sb[:, k : k + 1],
                                op0=ALU.mult, op1=ALU.add)
                        nc.sync.dma_start(
                            out=out_flat[k][:, b * FB : (b + 1) * FB], in_=yn)

                    def sched(b=blk, mu_s=mu_s, yb=y_blk, y2l=y2):
                        box = {}

                        def s1():
                            box["var"] = ln_ss_var(b, mu_s, y2l)

                        def s2():
                            box["rst"] = ln_rstd(b, box["var"])

                        def s2b():
                            ln_cast(b, mu_s, box["rst"])

                        def s3():
                            box["bc"] = ln_bcast(b)

                        defer(t + 1, s1)
                        defer(t + 2, s2)
                        defer(t + 3, s2b)
                        defer(t + 4, s3)
                        for k in range(HT):
                            defer(t + 5 + k,
                                  lambda k=k: ln_norm_k(b, k, yb[k], *box["bc"]))

                    sched()

                # -- final hidden state (chunk C-1 columns) --
                if t == T - 1:
                    for k in range(HT):
                        nc.sync.dma_start(
                            out=hl_d[k, :, :], in_=hn[k][:, C - 1 : S : C])

                # -- deferred LN work scheduled for this step --
                for fn in pending.pop(t, []):
                    fn()

            # -- flush LN work scheduled past the last step --
            for step in sorted(pending):
                for fn in pending[step]:
                    fn()
            pending.clear()
    nc.compile()
    return nc


def stage_inputs(input, h, is_initial, W_ih, W_hh, b_ih, b_hh, gamma, beta, R):
    """Host-side sharding/staging. Returns per-core input maps."""
    T = R + KS
    x = np.asarray(input, np.float32)
    h0 = np.asarray(h, np.float32)
    ii = np.asarray(is_initial).reshape(N, L)
    W_ih = np.asarray(W_ih, np.float32)
    W_hh = np.asarray(W_hh, np.float32)
    b_ih = np.asarray(b_ih, np.float32)
    b_hh = np.asarray(b_hh, np.float32)
    gamma = np.asarray(gamma, np.float32)
    beta = np.asarray(beta, np.float32)

    mask = 1.0 - ii.astype(np.float32)  # [N, L]

    # l index per (c, t): warm-up reads the R steps before the chunk;
    # chunk 0's warm-up reads l in [KS-R, KS) (discarded garbage).
    l_for = np.empty((C, T), np.int64)
    for c in range(C):
        for t in range(T):
            l = c * KS + (t - R)
            l_for[c, t] = l if l >= 0 else l + KS
    # masks per phase step t: state entering step t is h * ms[t]
    # ms[t, s] with s = n*C + c uses mask at l_for[c, t]
    # (for t=0 it is unused; state starts at zero)

    # weight layouts: wih[k, p, g] = W_ih[g, k*128+p]
    wihT = np.ascontiguousarray(
        W_ih.T.reshape(HT, 128, 3 * H)).astype(BF_NP)
    whhT = np.ascontiguousarray(
        W_hh.T.reshape(HT, 128, 3 * H)).astype(BF_NP)
    brz = (b_ih + b_hh)[: 2 * H].reshape(8, 128).T.copy()        # [128, 8]
    bhn = b_hh[2 * H :].reshape(HT, 128).T.copy()                # [128, 4]
    binn = b_ih[2 * H :].reshape(HT, 128).T.copy()
    gam = gamma.reshape(HT, 128).T.copy()
    bet = beta.reshape(HT, 128).T.copy()
    ones = np.ones((128, 1), BF_NP)
    zeros = np.zeros((128, S), BF_NP)

    in_maps = []
    for core in range(NCORES):
        n0 = core * NB
        xc = x[n0 : n0 + NB]              # [NB, L, H]
        # xs[k, p, t, s] = x[n, l_for[c, t], k*128+p], s = n*C + c
        xg = xc[:, l_for, :]              # [NB, C, T, H]
        xs = np.ascontiguousarray(
            xg.transpose(3, 2, 0, 1).reshape(HT, 128, T, S)).astype(BF_NP)
        mg = mask[n0 : n0 + NB][:, l_for]  # [NB, C, T]
        ms = np.ascontiguousarray(
            mg.transpose(2, 0, 1).reshape(T, S)).astype(BF_NP)
        m0 = mask[n0 : n0 + NB, 0]         # [NB]
        h0m = np.ascontiguousarray(
            (h0[n0 : n0 + NB] * m0[:, None]).T.reshape(HT, 128, NB)).astype(BF_NP)
        in_maps.append({
            "xs": xs, "ms": ms, "h0m": h0m,
            "wih": wihT, "whh": whhT, "brz": brz, "bhn": bhn, "bin": binn,
            "gam": gam, "bet": bet, "ones": ones, "zeros": zeros,
        })
    return in_maps


def required_warmup(is_initial):
    """Max distance from a chunk boundary back to the latest reset."""
    ii = np.asarray(is_initial).reshape(N, L)
    need = 0
    for c in range(1, C):
        start = c * KS
        sub = ii[:, :start]
        for n in range(N):
            nz = np.nonzero(sub[n])[0]
            gap = start - nz[-1] if len(nz) else start
            need = max(need, gap)
    return need


def unstage_outputs(results):
    out = np.empty((N, L, H), np.float32)
    h_last = np.empty((N, H), np.float32)
    for core in range(NCORES):
        n0 = core * NB
        st = np.asarray(results[core]["out_st"], dtype=np.float32)  # [HT,128,KS,S]
        o = st.reshape(HT, 128, KS, NB, C).transpose(3, 4, 2, 0, 1)
        out[n0 : n0 + NB] = o.reshape(NB, L, H)
        hl = np.asarray(results[core]["hlast"], dtype=np.float32)   # [HT,128,NB]
        h_last[n0 : n0 + NB] = hl.transpose(2, 0, 1).reshape(NB, H)
    h_exp = np.broadcast_to(h_last[:, None, :], (N, L, H)).copy()
    return out, h_exp


_PROGRAM_CACHE = {}


def kernel(input, h, is_initial, W_ih, W_hh, b_ih, b_hh, gamma, beta):
    need = required_warmup(is_initial)
    R = max(4, need)
    triv = bool(
        np.all(np.asarray(gamma) == 1.0) and np.all(np.asarray(beta) == 0.0))
    key = (R, triv)
    if key not in _PROGRAM_CACHE:
        _PROGRAM_CACHE[key] = build_program(R, triv_gb=triv)
    nc = _PROGRAM_CACHE[key]
    in_maps = stage_inputs(
        input, h, is_initial, W_ih, W_hh, b_ih, b_hh, gamma, beta, R)
    res = run_bass_kernel_spmd(nc, in_maps, list(range(NCORES))).results
    return unstage_outputs(res)
